# revision 21
# baseline (speedup 1.0000x reference)
"""Trainium2 Bass kernel for DeformationNetworkGraphConvolutionalFullRes.

Full (unsharded) inputs in, full output out. Data-parallel over the 4 meshes:
core m processes mesh m (cores 4-7 idle). Inside each core:

  - vert_align sampling as (S @ F) @ W == S @ (F @ W): per feature map,
    F[C,HW] @ Wslice[C,128] -> G[HW,128] (bf16), then the sparse bilinear
    operator S applied as dense [128px, 512vert] bf16 blocks (built host-side)
    streamed into the TensorEngine, accumulating over (map, pixel-tile) pairs
    in PSUM. Vertices pre-sorted by image cell to localize pixel tiles.
  - GraphConv layers: h1 = x@W1 rows written to HBM in a partition-major
    row numbering (so the SBUF->HBM writes use 1792B descriptors); messages
    h1[src] pulled with dma_gather in dst-sorted edge order, packed per
    4-tile dst group (padding only at group tail); the segmented sum over
    edges is done with PRECOMPUTED fp8 one-hot blocks (resident in SBUF,
    shared by all 8 layers) as matmuls accumulating in PSUM on top of
    h0 = x@W0 (+ rank-1 image-encoding term); ReLU writes the transposed
    bf16 activations for the next layer directly.
"""

import ml_dtypes
import numpy as np
from contextlib import ExitStack

import concourse.bass as bass
import concourse.tile as tile
from concourse import bacc, mybir
from concourse.bass_utils import run_bass_kernel_spmd

# ---------------- problem constants (hardcoded per spec) ----------------
B = 4
V = 10242
E_PER = 30720
HID = 128
MAPS = [(256, 56), (512, 28), (1024, 14), (2048, 7)]  # (C, H==W)
CH_OFF = [0, 256, 768, 1792, 3840]

VP = 10752            # padded vertex count: 84 tiles of 128
NT = VP // 128        # 84 vertex tiles
NVCH = VP // 512      # 21 vertex chunks (sampling)
GT = 4                # dst tiles per scatter group
NGRP = NT // GT       # 21 groups
GW = GT * 128         # 512 dst slots per group
SPLIT_GRP = 11        # h1-write batches the part-A gathers depend on
K_PRE = 4             # leading groups whose part-A gather is pre-emitted

F32 = mybir.dt.float32
BF16 = mybir.dt.bfloat16
FP8 = mybir.dt.float8e4
I16 = mybir.dt.int16
AF = mybir.ActivationFunctionType
BFNP = ml_dtypes.bfloat16
F8NP = ml_dtypes.float8_e4m3fn


def _corners(grid, W):
    """grid [V,2] in [-1,1] -> list of (pix_idx int64, weight f32) per corner."""
    x = (grid[:, 0] + 1.0) * 0.5 * (W - 1)
    y = (grid[:, 1] + 1.0) * 0.5 * (W - 1)
    x0f, y0f = np.floor(x), np.floor(y)
    wx1, wy1 = (x - x0f).astype(np.float32), (y - y0f).astype(np.float32)
    wx0, wy0 = 1.0 - wx1, 1.0 - wy1
    x0 = np.clip(x0f, 0, W - 1).astype(np.int64)
    x1 = np.clip(x0f + 1, 0, W - 1).astype(np.int64)
    y0 = np.clip(y0f, 0, W - 1).astype(np.int64)
    y1 = np.clip(y0f + 1, 0, W - 1).astype(np.int64)
    return [
        (y0 * W + x0, wy0 * wx0),
        (y0 * W + x1, wy0 * wx1),
        (y1 * W + x0, wy1 * wx0),
        (y1 * W + x1, wy1 * wx1),
    ]


def _prep(inputs):
    """Host-side restructuring: sorting, padding, index tables, sparse-operator
    blocks, fp8 one-hot scatter blocks. Returns (cfg, per_core_aux, post)."""
    feats = [inputs["feat1"], inputs["feat2"], inputs["feat3"], inputs["feat4"]]
    av = np.asarray(inputs["aligned_verts"], np.float32)
    verts = np.asarray(inputs["verts_packed"], np.float32)
    enc = np.asarray(inputs["image_enc"], np.float32)
    edges = np.asarray(inputs["edges"], np.int64)

    for bn in ["bottleneck_b", "g0_b0", "g0_b1", "off_b"]:
        assert not np.any(np.asarray(inputs[bn])), f"{bn} nonzero: unsupported"
    assert not np.any(np.asarray(inputs["gb0"])) and not np.any(
        np.asarray(inputs["gb1"])
    ), "gb nonzero: unsupported"

    # per-mesh vertex sort (by finest-map cell) ----------------------------
    sigmas, invs, corners_all = [], [], []
    for m in range(B):
        grid = av[m, :, :2]
        cs = _corners(grid, MAPS[0][1])
        key = cs[0][0]  # y0*56+x0 of map 0
        sigma = np.argsort(key, kind="stable")
        inv = np.empty(V, np.int64)
        inv[sigma] = np.arange(V)
        sigmas.append(sigma)
        invs.append(inv)
        corners_all.append(
            [[(pix[sigma], w[sigma]) for (pix, w) in _corners(grid, Wm)]
             for (_, Wm) in MAPS]
        )

    # sampling schedule: per (map, vchunk) the union over meshes of touched
    # pixel tiles ---------------------------------------------------------
    ntile_map = [(Wm * Wm + 127) // 128 for (_, Wm) in MAPS]
    g_off = np.cumsum([0] + ntile_map)  # global G-tile offsets
    sched = []
    for mi in range(4):
        per_c = []
        for c in range(NVCH):
            lo, hi = c * 512, min((c + 1) * 512, V)
            tiles = set()
            if lo < V:
                for m in range(B):
                    for (pix, _w) in corners_all[m][mi]:
                        pc = pix[lo:hi] // 128
                        tiles.update(np.unique(pc).tolist())
            per_c.append(sorted(tiles) if tiles else [0])
        np_m = max(len(t) for t in per_c)
        per_c = [t + [t[0]] * (np_m - len(t)) for t in per_c]  # pad (zero blocks)
        sched.append(per_c)
    np_list = [len(sched[mi][0]) for mi in range(4)]
    npair = sum(np_list) * NVCH

    # graph structure ------------------------------------------------------
    # directed edges in sorted-vertex space, packed per dst group of GT
    # tiles; within a group, edges with src tile < SPLIT_T come first (their
    # gather only depends on the first SPLIT_GRP h1-write batches, so it can
    # run during the layer tail); each part is sorted by (dst, src).
    SPLIT_T = SPLIT_GRP * GT
    es = []
    grp_cnt = np.zeros((B, NGRP), np.int64)
    grp_cntA = np.zeros((B, NGRP), np.int64)
    grp_edges = []  # per mesh: list over groups of (dst_g, src_g) arrays
    for m in range(B):
        e = edges[m * E_PER:(m + 1) * E_PER] - m * V
        a = invs[m][e[:, 0]]
        b = invs[m][e[:, 1]]
        dst = np.concatenate([a, b])
        src = np.concatenate([b, a])
        part = ((src // 128 >= SPLIT_T) & (dst // GW < K_PRE)).astype(np.int64)
        order = np.lexsort((src, dst, part, dst // GW))
        dst, src, part = dst[order], src[order], part[order]
        i0 = np.searchsorted(dst // GW, np.arange(NGRP), side="left")
        i1 = np.searchsorted(dst // GW, np.arange(NGRP) + 1, side="left")
        grp_edges.append([(dst[i0[g]:i1[g]], src[i0[g]:i1[g]])
                          for g in range(NGRP)])
        grp_cnt[m] = i1 - i0
        for g in range(NGRP):
            grp_cntA[m, g] = int((part[i0[g]:i1[g]] == 0).sum())
    nidx = grp_cnt.max(axis=0)                        # max edges per group
    ngs = np.maximum(1, -(-nidx // 128)).astype(int)  # subchunks per group
    sub_base = np.concatenate([[0], np.cumsum(ngs)]).astype(int)
    tot_sub = int(sub_base[-1])
    kbA = (grp_cntA // 128).min(axis=0).astype(int)   # pure-A subchunks

    # per-subchunk dst-tile span (union over meshes); part-boundary
    # subchunks may span up to GT tiles
    tile_lo = np.full(tot_sub, GT, np.int64)
    tile_hi = np.full(tot_sub, -1, np.int64)
    for m in range(B):
        for g in range(NGRP):
            dg = grp_edges[m][g][0] - g * GW
            for k in range(ngs[g]):
                seg = dg[k * 128:(k + 1) * 128]
                if len(seg) == 0:
                    continue
                s = sub_base[g] + k
                tile_lo[s] = min(tile_lo[s], seg.min() // 128)
                tile_hi[s] = max(tile_hi[s], seg.max() // 128)
    empty = tile_hi < 0
    tile_lo[empty] = 0
    tile_hi[empty] = 0
    wid_t = (tile_hi - tile_lo + 1).astype(int)       # tiles per subchunk
    oh_off = np.concatenate([[0], np.cumsum(wid_t * 128)]).astype(int)
    total_w = int(oh_off[-1])

    # h1 row renumbering: v -> (v%128)*NT + v//128 (partition-major rows so
    # the h1 SBUF->HBM write is 1792B-contiguous per partition)
    vv = np.arange(VP)
    rn = (vv % 128) * NT + vv // 128

    per_core = []
    for m in range(B):
        slots = np.zeros(tot_sub * 128, np.int64)     # idx 0 = safe pad row
        oh = np.zeros((128, total_w), np.float32)
        for g in range(NGRP):
            dst_g, src_g = grp_edges[m][g]
            cnt = len(dst_g)
            so = sub_base[g] * 128
            slots[so:so + cnt] = rn[src_g]
            dg = dst_g - g * GW
            r = np.arange(cnt)
            s = sub_base[g] + r // 128
            p = r % 128
            col = oh_off[s] + (dg // 128 - tile_lo[s]) * 128 + dg % 128
            oh[p, col] = 1.0
        srcw = np.tile(slots.reshape(-1, 16).T, (8, 1)).astype(np.int16)

        # sampling blocks ---------------------------------------------------
        wsc = np.zeros((npair, 128, 512), np.float32)
        pi = 0
        for c in range(NVCH):
            lo, hi = c * 512, min((c + 1) * 512, V)
            for mi in range(4):
                seen = set()
                for t in sched[mi][c]:
                    blk = wsc[pi]
                    if lo < V and t not in seen:  # pad repeats stay zero
                        seen.add(t)
                        for (pix, w) in corners_all[m][mi]:
                            px = pix[lo:hi]
                            sel = (px >= t * 128) & (px < (t + 1) * 128)
                            jj = np.nonzero(sel)[0]
                            np.add.at(blk, (px[jj] - t * 128, jj), w[lo:hi][jj])
                    pi += 1
        assert pi == npair

        vt = np.zeros((3, VP), np.float32)
        vt[:, :V] = verts[m * V:(m + 1) * V][sigmas[m]].T

        aux = {
            "f1": feats[0][m].reshape(256, -1).astype(BFNP),
            "f2": feats[1][m].reshape(512, -1).astype(BFNP),
            "f3": feats[2][m].reshape(1024, -1).astype(BFNP),
            "f4": feats[3][m].reshape(2048, -1).astype(BFNP),
            "bw": np.asarray(inputs["bottleneck_w"]).astype(BFNP),
            "wsc": wsc.reshape(npair * 128, 512).astype(BFNP),
            "srcw": np.ascontiguousarray(srcw),
            "oh": oh.astype(F8NP),
            "vertsT": vt.astype(BFNP),
            "encc": enc[m].reshape(2, 128).T.copy().astype(BFNP),  # [128,2]
            "g0w0m": np.asarray(inputs["g0_w0"][:128]).astype(BFNP),
            "g0w0v": np.asarray(inputs["g0_w0"][128:131]).astype(BFNP),
            "g0w0e": np.ascontiguousarray(
                np.asarray(inputs["g0_w0"][131:387])).astype(BFNP),
            "g0w1m": np.asarray(inputs["g0_w1"][:128]).astype(BFNP),
            "g0w1v": np.asarray(inputs["g0_w1"][128:131]).astype(BFNP),
            "g0w1e": np.ascontiguousarray(
                np.asarray(inputs["g0_w1"][131:387])).astype(BFNP),
            "gw0": np.ascontiguousarray(
                np.asarray(inputs["gw0"], np.float32).transpose(1, 0, 2)
                .reshape(128, 7 * 128)).astype(BFNP),
            "gw1": np.ascontiguousarray(
                np.asarray(inputs["gw1"], np.float32).transpose(1, 0, 2)
                .reshape(128, 7 * 128)).astype(BFNP),
            "offw": np.asarray(inputs["off_w"]).astype(BFNP),
        }
        per_core.append(aux)

    cfg = {"sched": sched, "np_list": np_list, "npair": npair,
           "g_off": g_off.tolist(), "ntile_map": ntile_map,
           "ngs": ngs.tolist(), "sub_base": sub_base.tolist(),
           "tot_sub": tot_sub, "tile_lo": tile_lo.tolist(),
           "wid_t": wid_t.tolist(), "oh_off": oh_off.tolist(),
           "total_w": total_w, "kbA": kbA.tolist()}
    post = {"sigmas": sigmas}
    return cfg, per_core, post


def _build(cfg, shapes, dump=None, nlayers=8, repeat=1):
    """Build the SPMD Bass program (same instruction stream for all cores)."""
    nc = bacc.Bacc("TRN2", target_bir_lowering=False, debug=False, num_devices=B)
    ap = {}
    for name, arr in shapes.items():
        ap[name] = nc.dram_tensor(
            name, list(arr.shape), mybir.dt.from_np(arr.dtype),
            kind="ExternalInput").ap()
    out = nc.dram_tensor("out", [VP, 3], F32, kind="ExternalOutput").ap()
    xdump = (nc.dram_tensor("xdump", [128, VP], F32, kind="ExternalOutput").ap()
             if dump else None)
    h1d2 = [nc.dram_tensor("h1da", [VP, HID], BF16).ap(),
            nc.dram_tensor("h1db", [VP, HID], BF16).ap()]

    sched = cfg["sched"]
    g_off = cfg["g_off"]
    ntile_map = cfg["ntile_map"]
    NGT = g_off[4]
    ngs = cfg["ngs"]
    sub_base = cfg["sub_base"]
    tot_sub = cfg["tot_sub"]
    tile_lo = cfg["tile_lo"]
    wid_t = cfg["wid_t"]
    oh_off = cfg["oh_off"]
    total_w = cfg["total_w"]
    kbA = cfg["kbA"]
    SUBG = max(ngs)

    with tile.TileContext(nc) as tc, ExitStack() as ctx:
        # ---------------- persistent pools ----------------
        pp = ctx.enter_context(tc.tile_pool(name="pers", bufs=1))
        xa = pp.tile([128, VP], BF16, tag="xa")
        xb = pp.tile([128, VP], BF16, tag="xb")
        srcw_t = pp.tile([128, tot_sub * 8], I16, tag="srcw")
        oh_t = pp.tile([128, total_w], FP8, tag="oh")
        w0_t = pp.tile([128, 7 * 128], BF16, tag="w0")
        w1_t = pp.tile([128, 7 * 128], BF16, tag="w1")
        g0_t = pp.tile([128, 6 * 128], BF16, tag="g0")  # w0m,w1m,w0e(2),w1e(2)
        g0v_t = pp.tile([3, 256], BF16, tag="g0v")      # w0v, w1v
        offw_t = pp.tile([128, 3], BF16, tag="offw")
        ones_t = pp.tile([1, GW], BF16, tag="ones")
        erow_t = pp.tile([1, 256], BF16, tag="erow")    # e0row, e1row
        encc_t = pp.tile([128, 2], BF16, tag="encc")

        nc.sync.dma_start(srcw_t[:], ap["srcw"][:])
        nc.sync.dma_start(oh_t[:], ap["oh"][:])
        nc.sync.dma_start(w0_t[:], ap["gw0"][:])
        nc.sync.dma_start(w1_t[:], ap["gw1"][:])
        nc.sync.dma_start(g0_t[:, 0:128], ap["g0w0m"][:])
        nc.sync.dma_start(g0_t[:, 128:256], ap["g0w1m"][:])
        nc.sync.dma_start(
            g0_t[:, 256:512].rearrange("p (c h) -> p c h", h=128),
            ap["g0w0e"].rearrange("(c p) h -> p c h", p=128))
        nc.sync.dma_start(
            g0_t[:, 512:768].rearrange("p (c h) -> p c h", h=128),
            ap["g0w1e"].rearrange("(c p) h -> p c h", p=128))
        nc.sync.dma_start(g0v_t[:, 0:128], ap["g0w0v"][:])
        nc.sync.dma_start(g0v_t[:, 128:256], ap["g0w1v"][:])
        nc.sync.dma_start(offw_t[:], ap["offw"][:])
        nc.vector.memset(ones_t[:], 1.0)
        nc.sync.dma_start(encc_t[:], ap["encc"][:])

        psA = ctx.enter_context(tc.tile_pool(name="psA", bufs=2, space="PSUM"))

        # enc rank-1 rows: e{0,1} = g0_w{0,1}[131:387].T @ enc  -> [1,128]
        for k in range(2):
            pe = psA.tile([1, 128], F32, tag="p1")
            for cchunk in range(2):
                nc.tensor.matmul(
                    out=pe[:],
                    lhsT=encc_t[:, cchunk:cchunk + 1],
                    rhs=g0_t[:, 256 + k * 256 + cchunk * 128:
                             256 + k * 256 + cchunk * 128 + 128],
                    start=(cchunk == 0), stop=(cchunk == 1))
            nc.scalar.activation(erow_t[:, k * 128:(k + 1) * 128], pe[:],
                                 AF.Copy)

        lp = ctx.enter_context(tc.tile_pool(name="lay", bufs=3))
        lph = ctx.enter_context(tc.tile_pool(name="layh", bufs=2))
        lpv = ctx.enter_context(tc.tile_pool(name="layv", bufs=2))
        psx = ctx.enter_context(tc.tile_pool(name="psumx", bufs=3, space="PSUM"))

        sp = ctx.enter_context(tc.tile_pool(name="samp", bufs=1))
        spf = ctx.enter_context(tc.tile_pool(name="sampf", bufs=2))
        spw = ctx.enter_context(tc.tile_pool(name="sampw", bufs=2))
        spp1 = ctx.enter_context(tc.tile_pool(name="samppsum1", bufs=2,
                                              space="PSUM"))

        # zero the msg buffers once (padding slots must stay finite: their
        # one-hot columns are zero, but PE 0*NaN would poison PSUM)
        MSGB = 3
        for _ in range(MSGB):
            mz = lp.tile([128, SUBG, 128], BF16, tag="msg")
            nc.vector.memset(mz[:].rearrange("p s h -> p (s h)"), 0.0)

        def _h1_group(l, g, cur, h1_writes):
            """h1_{l} for group g's tiles (from x_l columns) -> h1d[l%2].
            Rows of h1d are partition-major: row (p*NT + t)."""
            h1d = h1d2[l % 2]
            hst = lph.tile([128, GT * 128], BF16, tag="hst")
            if l == 0:
                vv = lpv.tile([3, GW], BF16, tag="vt")
                nc.scalar.dma_start(vv[:], ap["vertsT"][:, g * GW:(g + 1) * GW])
            for ti in range(GT):
                t = g * GT + ti
                ph = psA.tile([128, 128], F32, tag="p1")
                if l == 0:
                    nc.tensor.matmul(
                        out=ph[:], lhsT=cur[:, t * 128:(t + 1) * 128],
                        rhs=g0_t[:, 128:256], start=True, stop=False)
                    nc.tensor.matmul(
                        out=ph[:], lhsT=vv[:, ti * 128:(ti + 1) * 128],
                        rhs=g0v_t[:, 128:256], start=False, stop=False)
                    nc.tensor.matmul(
                        out=ph[:], lhsT=ones_t[:, 0:128],
                        rhs=erow_t[:, 128:256], start=False, stop=True)
                else:
                    nc.tensor.matmul(
                        out=ph[:], lhsT=cur[:, t * 128:(t + 1) * 128],
                        rhs=w1_t[:, (l - 1) * 128:l * 128],
                        start=True, stop=True)
                nc.scalar.activation(hst[:, ti * 128:(ti + 1) * 128],
                                     ph[:], AF.Copy)
            h1_writes.append(nc.scalar.dma_start(
                h1d.rearrange("(p n) c -> p n c", p=128)[:, g * GT:(g + 1) * GT, :],
                hst[:].rearrange("p (n c) -> p n c", c=128)))

        def _once(emit_out):
            # ---------------- phase 1: sampling ----------------
            g_sb = sp.tile([128, NGT * 128], BF16, tag="gsb")
            for mi, (C, Wm) in enumerate(MAPS):
                HW = Wm * Wm
                ncc = C // 128
                bw_t = spf.tile([128, 16 * 128], BF16, tag="bw")
                nc.sync.dma_start(
                    bw_t[:, :ncc * 128].rearrange("p (c h) -> p c h", h=128),
                    ap["bw"].rearrange("(c p) h -> p c h", p=128)
                    [:, CH_OFF[mi] // 128:CH_OFF[mi] // 128 + ncc, :])
                fm_t = sp.tile([128, 2 * 3136], BF16, tag="fm")
                nc.sync.dma_start(
                    fm_t[:, :ncc * HW].rearrange("p (c hw) -> p c hw", c=ncc),
                    ap[f"f{mi+1}"].rearrange("(c p) hw -> p c hw", p=128))
                for t in range(ntile_map[mi]):
                    p0 = t * 128
                    pcnt = min(128, HW - p0)
                    pg = psA.tile([128, 128], F32, tag="p1")
                    for cc in range(ncc):
                        nc.tensor.matmul(
                            out=pg[:pcnt, :],
                            lhsT=fm_t[:, cc * HW + p0:cc * HW + p0 + pcnt],
                            rhs=bw_t[:, cc * 128:cc * 128 + 128],
                            start=(cc == 0), stop=(cc == ncc - 1))
                    gt = g_off[mi] + t
                    nc.scalar.activation(
                        g_sb[:pcnt, gt * 128:gt * 128 + 128], pg[:pcnt, :],
                        AF.Copy)

            npc = sum(len(sched[mi][0]) for mi in range(4))  # pairs per chunk
            writes0 = []
            for c in range(NVCH):
                ps = spp1.tile([128, 512], F32, tag="ps")
                pairs_c = []
                for mi in range(4):
                    for t in sched[mi][c]:
                        pairs_c.append((mi, t))
                assert len(pairs_c) == npc
                half = (npc + 1) // 2
                wts = []
                for hb in range(2):
                    k0, k1 = hb * half, min((hb + 1) * half, npc)
                    wt = spw.tile([128, half, 512], BF16, tag="wsc")
                    nc.sync.dma_start(
                        wt[:, :k1 - k0, :],
                        ap["wsc"].rearrange("(k p) h -> p k h", p=128)
                        [:, c * npc + k0:c * npc + k1, :])
                    wts.append(wt)
                for k, (mi, t) in enumerate(pairs_c):
                    HW = MAPS[mi][1] ** 2
                    pcnt = min(128, HW - t * 128)
                    gt = g_off[mi] + t
                    nc.tensor.matmul(
                        out=ps[:],
                        lhsT=g_sb[:pcnt, gt * 128:gt * 128 + 128],
                        rhs=wts[k // half][:pcnt, k % half, :],
                        start=(k == 0), stop=(k == len(pairs_c) - 1))
                nc.scalar.activation(xa[:, c * 512:(c + 1) * 512], ps[:],
                                     AF.Relu)
                # chunk c == group c: emit layer-0 h1 for these 4 tiles now
                _h1_group(0, c, xa, writes0)

            # ---------------- phase 2: graph conv layers ----------------
            def _gather(h1d, g, msg, k0, k1, dep_writes):
                s0 = sub_base[g]
                gi = nc.gpsimd.dma_gather(
                    out_ap=msg[:, k0:k1, :],
                    in_ap=h1d[:],
                    idxs_ap=srcw_t[:, (s0 + k0) * 8:(s0 + k1) * 8],
                    num_idxs=(k1 - k0) * 128,
                    num_idxs_reg=(k1 - k0) * 128,
                    elem_size=HID,
                    single_packet=False,
                )
                for wi in dep_writes:
                    tile.add_dep_helper(gi.ins, wi.ins,
                                        reason="h1 RAW: gather after write")

            def _pre_gathers(l, writes):
                """Emit part-A gathers of layer l's first K_PRE groups; they
                only depend on the first SPLIT_GRP h1-write batches, so they
                fill the DMA idle in the previous layer's tail."""
                pre = {}
                for g in range(K_PRE):
                    msg = lp.tile([128, SUBG, 128], BF16, tag="msg")
                    if kbA[g] > 0:
                        _gather(h1d2[l % 2], g, msg, 0, kbA[g],
                                writes[:SPLIT_GRP])
                    pre[g] = msg
                return pre

            cur, nxt = xa, xb
            h1_writes = writes0
            pre = _pre_gathers(0, writes0)
            for l in range(nlayers):
                h1d = h1d2[l % 2]
                next_writes = []
                for g in range(NGRP):
                    s0, s1 = sub_base[g], sub_base[g + 1]
                    ng = s1 - s0
                    if g in pre:
                        msg = pre.pop(g)
                        if kbA[g] < ng:
                            _gather(h1d, g, msg, kbA[g], ng, h1_writes)
                    else:
                        msg = lp.tile([128, SUBG, 128], BF16, tag="msg")
                        _gather(h1d, g, msg, 0, ng, h1_writes)
                    if l == 0:
                        vv2 = lpv.tile([3, GW], BF16, tag="vt2")
                        nc.scalar.dma_start(
                            vv2[:], ap["vertsT"][:, g * GW:(g + 1) * GW])
                    px = psx.tile([128, GW], F32, tag="px")
                    if l == 0:
                        nc.tensor.matmul(
                            out=px[:], lhsT=g0_t[:, 0:128],
                            rhs=cur[:, g * GW:(g + 1) * GW],
                            start=True, stop=False)
                        nc.tensor.matmul(
                            out=px[:], lhsT=g0v_t[:, 0:128],
                            rhs=vv2[:], start=False, stop=False)
                        nc.tensor.matmul(
                            out=px[:], lhsT=erow_t[:, 0:128],
                            rhs=ones_t[:], start=False, stop=False)
                    else:
                        nc.tensor.matmul(
                            out=px[:], lhsT=w0_t[:, (l - 1) * 128:l * 128],
                            rhs=cur[:, g * GW:(g + 1) * GW],
                            start=True, stop=False)
                    for k in range(ng):
                        s = s0 + k
                        co = tile_lo[s] * 128
                        w = wid_t[s] * 128
                        nc.tensor.matmul(
                            out=px[:, co:co + w],
                            lhsT=msg[:, k, :],
                            rhs=oh_t[:, oh_off[s]:oh_off[s] + w],
                            start=False, stop=(k == ng - 1),
                            skip_group_check=True)
                    nc.scalar.activation(nxt[:, g * GW:(g + 1) * GW], px[:],
                                         AF.Relu)
                    # overlap next layer's h1 for this group with the
                    # remaining gathers of the current layer
                    if l + 1 < nlayers:
                        _h1_group(l + 1, g, nxt, next_writes)
                    elif emit_out:
                        # last layer: emit this group's output rows now
                        ost = lph.tile([128, GT * 3], F32, tag="ost")
                        for ti in range(GT):
                            t = g * GT + ti
                            po = psA.tile([128, 3], F32, tag="p1")
                            nc.tensor.matmul(
                                out=po[:], lhsT=nxt[:, t * 128:(t + 1) * 128],
                                rhs=offw_t[:], start=True, stop=True)
                            nc.scalar.activation(ost[:, ti * 3:(ti + 1) * 3],
                                                 po[:], AF.Copy)
                        nc.scalar.dma_start(
                            out.rearrange("(n p) c -> p n c", p=128)
                            [:, g * GT:(g + 1) * GT, :],
                            ost[:].rearrange("p (n c) -> p n c", c=3))
                if l + 1 < nlayers:
                    pre = _pre_gathers(l + 1, next_writes)
                cur, nxt = nxt, cur
                h1_writes = next_writes

        for _rep in range(repeat):
            _once(_rep == repeat - 1)
        cur = xa if nlayers % 2 == 0 else xb

        if xdump is not None:
            nc.sync.dma_start(xdump[:], cur[:])

    nc.compile()
    return nc


_CACHE = {}


def kernel(**inputs) -> np.ndarray:
    cfg, per_core, post = _prep(inputs)
    key = (cfg["npair"], tuple(cfg["np_list"]), cfg["tot_sub"], cfg["total_w"])
    if key not in _CACHE:
        _CACHE[key] = _build(cfg, per_core[0])
    nc = _CACHE[key]
    res = run_bass_kernel_spmd(nc, per_core, list(range(B)))
    outs = np.empty((B, V, 3), np.float32)
    for m in range(B):
        rows = res.results[m]["out"][:V]
        outs[m][post["sigmas"][m]] = rows
    return outs.reshape(B * V, 3)


if __name__ == "__main__":
    pass
